# revision 19
# baseline (speedup 1.0000x reference)
"""CUR attention (Nystrom-style) Trainium2 kernel, v2.

Full inputs Q,K,V [8, 8, 4096, 64] f32 + mask [8, 4096] bool; output X same
shape as Q. Sharded batch-per-core across 8 NeuronCores; each core handles
its batch's 8 heads.

Math (per (b,h), N=4096, D=64, M=64):
  scores_K = K.sum(-1); idxK = top-64      -> nc = K[idxK]
  scores_Q = Q.sum(-1); idxQ = top-64      -> nr = Q[idxQ]/8
  kernel_1 = softmax(Q/8 @ nc.T, -1)            [N, M]
  u        = softmax(nr @ nc.T, -1)             [M, M]  (rows of kernel_1 at idxQ)
  kernel_3 = softmax(nr @ K.T, -1)              [M, N]
  X = kernel_1 @ (newton_schulz_inv(u) @ (kernel_3 @ V))

v2 structure (vs v1): depth-2 software-pipelined head loop — iteration h
emits load(h+2) DMAs, the selection chain for head h+1 (sliced into stages
interleaved between heavy 4-chunk groups so no engine queue stalls), and the
heavy transpose/logit/exp/RV pipeline for head h.  The [1,1] AllReduce-max
(Newton-Schulz init needs the GLOBAL max of u-colsums) fires as soon as
head 7's u lands, overlapping the last heavy phases.  Newton-Schulz runs in
bf16 (validated 9.5e-3 rel err vs 2e-2 budget), X is produced per head-PAIR
(contraction packs two heads' 64-dim m axes into one 128-part matmul) and
stored bf16; kernel() upcasts/reorders on host.

Top-64 selection: exact threshold tau = 65th-largest score via two-level
pruning (per-chunk top-16 from a score transpose, then per-rank-row top-16
across chunks from a second transpose) and a rank-count over the surviving
256 candidates; bounds host-verified (`_selection_prune_safe`).  Landmark
order is sparse_gather scan order - a permutation that provably cancels in X.
Softmaxes skip max-subtraction: logits are dot products of unit-scale
gaussians (|logit| < ~7 << 88), exp cannot overflow in f32.
"""
import math
import numpy as np

import concourse.bacc as bacc
import concourse.bass as bass
import concourse.tile as tile
import concourse.mybir as mybir
from concourse._compat import with_exitstack
from concourse.bass_utils import run_bass_kernel_spmd
from concourse.masks import make_identity

F32 = mybir.dt.float32
BF16 = mybir.dt.bfloat16
I16 = mybir.dt.int16
U8 = mybir.dt.uint8
AF = mybir.ActivationFunctionType
ALU = mybir.AluOpType
AX = mybir.AxisListType

B, H, N, D, M = 8, 8, 4096, 64, 64
NT = N // 128          # 32 chunks of 128 rows
NG = NT // 4           # 8 groups of 4 chunks
N_ITER = 6


@with_exitstack
def cur_attention_body(ctx, tc, q, k, v, x, cc_in, cc_out, dbg=None):
    """q/k/v: DRAM APs [H, N, D] f32; x: DRAM AP [4, 128, NT, 2, 64] bf16.
    cc_in/cc_out: [1, 1] f32 DRAM APs for the AllReduce-max (cc_out Shared).
    """
    nc = tc.nc

    def dump(name, ap):
        if dbg is not None and name in dbg:
            nc.sync.dma_start(dbg[name], ap)

    const = ctx.enter_context(tc.tile_pool(name="const", bufs=1))
    ident = const.tile([128, 128], F32, tag="ident")
    make_identity(nc, ident)
    ident_bf = const.tile([64, 64], BF16, tag="ident_bf")
    nc.vector.tensor_copy(ident_bf[:], ident[0:64, 0:64])
    iota_f = const.tile([128, NT], F32, tag="iota_f")
    iota_i = const.tile([128, NT], mybir.dt.int32, tag="iota_i")
    nc.gpsimd.iota(iota_i[:], pattern=[[128, NT]], base=0, channel_multiplier=1)
    nc.vector.tensor_copy(iota_f[:], iota_i[:])
    ones_row = const.tile([1, 128], F32, tag="ones_row")
    nc.vector.memset(ones_row[:], 1.0)
    ones128 = const.tile([128, 128], F32, tag="ones128")
    nc.vector.memset(ones128[:], 1.0)
    rep16 = const.tile([16, 128], F32, tag="rep16")
    nc.vector.tensor_copy(
        rep16[:].rearrange("r (g c) -> r g c", g=8),
        ident[0:16, 0:16].rearrange("r c -> r () c").broadcast_to([16, 8, 16]))
    # batched aI tiles for Newton-Schulz: [64, H, 64] with a*I in each slot
    i7 = const.tile([64, H, 64], F32, tag="i7")
    i15 = const.tile([64, H, 64], F32, tag="i15")
    i13 = const.tile([64, H, 64], F32, tag="i13")
    for t_, val in ((i7, 7.0), (i15, 15.0), (i13, 13.0)):
        nc.gpsimd.memset(t_[:], 0.0)
        for p in range(H):
            nc.gpsimd.affine_select(
                out=t_[:, p, :], in_=t_[:, p, :],
                compare_op=ALU.not_equal, fill=val,
                base=0, pattern=[[-1, 64]], channel_multiplier=1)

    # ---- pools ----
    io = ctx.enter_context(tc.tile_pool(name="io", bufs=4))
    selp = ctx.enter_context(tc.tile_pool(name="selp", bufs=2))
    lm = ctx.enter_context(tc.tile_pool(name="lm", bufs=2))
    chunk = ctx.enter_context(tc.tile_pool(name="chunk", bufs=3))
    ect = ctx.enter_context(tc.tile_pool(name="ect", bufs=1))
    nsbuf = ctx.enter_context(tc.tile_pool(name="nsbuf", bufs=1))
    work = ctx.enter_context(tc.tile_pool(name="work", bufs=2))
    xo = ctx.enter_context(tc.tile_pool(name="xo", bufs=2))
    ps = ctx.enter_context(tc.tile_pool(name="ps", bufs=1, space="PSUM"))
    ps_t = ctx.enter_context(tc.tile_pool(name="ps_t", bufs=2, space="PSUM"))
    ps_c = ctx.enter_context(tc.tile_pool(name="ps_c", bufs=2, space="PSUM"))
    ps_r = ctx.enter_context(tc.tile_pool(name="ps_r", bufs=2, space="PSUM"))
    ps_acc = ctx.enter_context(tc.tile_pool(name="ps_acc", bufs=1, space="PSUM"))

    # cross-head state
    u_bf = nsbuf.tile([64, H, 64], BF16, tag="u_bf")
    uT_bf = nsbuf.tile([64, H, 64], BF16, tag="uT_bf")
    rvn_bf = nsbuf.tile([64, H, 64], BF16, tag="rvn_bf")
    csall = nsbuf.tile([64, H], F32, tag="csall")
    # exp_cT pair tiles [128, NT, 128] bf16: even head in partitions 0:64,
    # odd head in 64:128
    ect_tiles = [ect.tile([128, NT, 128], BF16, tag=f"ect{pr}",
                          name=f"ect{pr}") for pr in range(4)]

    qk_t, vext_t, s2_t = {}, {}, {}
    sel_state = {}

    def emit_load(h):
        qk = io.tile([128, NT, 128], F32, tag="qk")
        nc.sync.dma_start(qk[:, :, 0:64],
                          q[h].rearrange("(t p) d -> p t d", p=128))
        nc.sync.dma_start(qk[:, :, 64:128],
                          k[h].rearrange("(t p) d -> p t d", p=128))
        vext = io.tile([128, NT, 65], BF16, tag="vext")
        nc.gpsimd.dma_start(vext[:, :, 0:64],
                            v[h].rearrange("(t p) d -> p t d", p=128))
        nc.vector.memset(vext[:, :, 64:65], 1.0)
        qk_t[h], vext_t[h] = qk, vext

    # ---------------- selection slices (head h) ----------------
    # sel index s: 0 = Q-selection, 1 = K-selection (qk col blocks q|k)
    def emit_sel_a(h):
        """scores + first transpose"""
        qk = qk_t[h]
        s2 = selp.tile([128, 2, NT], F32, tag="s2")
        nc.vector.tensor_reduce(
            s2[:], qk[:].rearrange("p t (s d) -> p s t d", s=2),
            axis=AX.X, op=ALU.add)
        sT_ps = ps.tile([64, 128], F32, tag="bank")
        nc.tensor.transpose(sT_ps[:], s2[:], ident[:])
        sT = selp.tile([64, 128], F32, tag="sT")
        nc.scalar.copy(sT[:], sT_ps[:])
        s2_t[h] = s2
        sel_state[h] = {"sT": sT}
        if h == 0:
            dump('s2', s2[:].rearrange("p s t -> p (s t)"))

    def emit_sel_b(h):
        """per-chunk top16 -> v1T -> v2 -> candrow -> cf/cb"""
        st = sel_state[h]
        sT = st["sT"]
        v1 = selp.tile([64, 16], F32, tag="v1")
        nc.vector.max(v1[:, 0:8], sT[:])
        nc.vector.match_replace(sT[:], in_to_replace=v1[:, 0:8],
                                in_values=sT[:], imm_value=-1e30)
        nc.vector.max(v1[:, 8:16], sT[:])
        # regroup: per sel, row r gets chunks {r, r+16} (2-chunk unions)
        w2 = selp.tile([16, 2, 32], F32, tag="w2")
        for s in range(2):
            nc.sync.dma_start(w2[:, s, 0:16], v1[32 * s:32 * s + 16, :])
            nc.sync.dma_start(w2[:, s, 16:32], v1[32 * s + 16:32 * s + 32, :])
        # v2 per sel: top-16 of each 2-chunk union row
        v2 = selp.tile([16, 32], F32, tag="v2")
        for s in range(2):
            cols = w2[:, s, :]
            nc.vector.max(v2[:, 16 * s:16 * s + 8], cols)
            nc.vector.match_replace(cols, in_to_replace=v2[:, 16 * s:16 * s + 8],
                                    in_values=cols, imm_value=-1e30)
            nc.vector.max(v2[:, 16 * s + 8:16 * s + 16], cols)
        crow = selp.tile([1, 512], F32, tag="crow")
        for s in range(2):
            nc.sync.dma_start(
                crow[:, 256 * s:256 * (s + 1)].rearrange("o (p c) -> o p c", p=16),
                v2[:, 16 * s:16 * (s + 1)])
        cb_ps = ps.tile([128, 512], F32, tag="bank")
        nc.tensor.matmul(cb_ps[:], ones_row[:], crow[:], start=True, stop=True)
        cb = selp.tile([128, 512], F32, tag="cb")
        nc.scalar.copy(cb[:], cb_ps[:])
        cf_ps = ps.tile([128, 4], F32, tag="bank")
        for s in range(2):
            for g in range(2):
                src = crow[:, 256 * s + g:256 * (s + 1):2]
                nc.tensor.transpose(cf_ps[:, 2 * s + g:2 * s + g + 1], src,
                                    ident[0:1, 0:1])
        cf = selp.tile([128, 4], F32, tag="cf")
        nc.vector.tensor_copy(cf[:], cf_ps[:])
        st.update(v2=v2, cb=cb, cf=cf)
        if h == 0:
            dump('v1', v1[:]); dump('v2', v2[:]); dump('cb', cb[:]); dump('cf', cf[:])

    def emit_sel_c(h):
        """rank count -> tau -> mask -> mi -> miT -> w16 -> sparse gather"""
        st = sel_state[h]
        cb, cf = st["cb"], st["cf"]
        s2 = s2_t[h]
        taub = {}
        for s in range(2):
            cmp = selp.tile([128, 2, 256], U8, tag=f"cmp{s}")
            nc.vector.tensor_tensor(
                cmp[:],
                cb[:, 256 * s:256 * (s + 1)]
                .rearrange("p c -> p () c").broadcast_to([128, 2, 256]),
                cf[:, 2 * s:2 * s + 2]
                .rearrange("p j -> p j ()").broadcast_to([128, 2, 256]),
                op=ALU.is_gt)
            rank = selp.tile([128, 2], F32, tag=f"rank{s}")
            nc.vector.tensor_reduce(rank[:], cmp[:], axis=AX.X, op=ALU.add)
            taupart = selp.tile([128, 2], F32, tag=f"taupart{s}")
            nc.vector.scalar_tensor_tensor(
                taupart[:], rank[:], 64.0, cf[:, 2 * s:2 * s + 2],
                op0=ALU.is_equal, op1=ALU.mult)
            taucol = selp.tile([128, 1], F32, tag=f"taucol{s}")
            nc.vector.tensor_reduce(taucol[:], taupart[:], axis=AX.X, op=ALU.add)
            taub_ps = ps.tile([128, 1], F32, tag="bank")
            nc.tensor.matmul(taub_ps[:], ones128[:], taucol[:],
                             start=True, stop=True)
            tb = selp.tile([128, 1], F32, tag=f"taub{s}")
            nc.vector.tensor_copy(tb[:], taub_ps[:])
            taub[s] = tb
        w16s = []
        for s in range(2):
            msk = selp.tile([128, NT], U8, tag=f"msk{s}")
            nc.vector.tensor_scalar(msk[:], s2[:, s, :], taub[s][:, 0:1], None,
                                    op0=ALU.is_gt)
            mi = selp.tile([128, NT], F32, tag=f"mi{s}")
            nc.vector.memset(mi[:], -1.0)
            nc.vector.copy_predicated(mi[:], msk[:], iota_f[:])
            miT_ps = ps.tile([32, 128], F32, tag="bank")
            nc.tensor.transpose(miT_ps[:], mi[:], ident[:])
            miT = selp.tile([32, 128], F32, tag=f"miT{s}")
            nc.scalar.copy(miT[:], miT_ps[:])
            w16 = selp.tile([16, 256], F32, tag=f"w16{s}")
            nc.sync.dma_start(w16[:, 0:128], miT[0:16, :])
            nc.sync.dma_start(w16[:, 128:256], miT[16:32, :])
            comp = selp.tile([16, 4], F32, tag=f"comp{s}")
            nf = selp.tile([1, 1], mybir.dt.uint32, tag=f"nf{s}")
            nc.gpsimd.sparse_gather(comp[:], w16[:], num_found=nf[:])
            w16s.append(comp)
            if h == 0:
                dump(f'comp{s}', comp[:])
        st.update(comp=w16s)

    def emit_sel_d(h):
        """idx replication -> dma_gather both sels"""
        st = sel_state[h]
        gsel = lm.tile([128, 128], F32, tag="gsel")
        for s, src in ((0, q), (1, k)):
            comp = st["comp"][s]
            rep_ps = ps.tile([128, 4], F32, tag="bank")
            nc.tensor.matmul(rep_ps[:], rep16[:], comp[:], start=True, stop=True)
            idxr = selp.tile([128, 4], I16, tag=f"idxr{s}")
            nc.vector.tensor_copy(idxr[:], rep_ps[:])
            nc.gpsimd.dma_gather(
                gsel[:, 64 * s:64 * (s + 1)].rearrange("p (a bb) -> p a bb", a=1),
                src[h], idxr[:], num_idxs=64, num_idxs_reg=64, elem_size=64)
        st["gsel"] = gsel
        if h == 0:
            dump('gsel', gsel[0:64, :])

    def emit_sel_e(h):
        """landmark transposes -> ncT8/nrT8z -> u -> uT -> colsums"""
        st = sel_state[h]
        gsel = st["gsel"]
        # gsel rows 0:64 = landmarks; cols 0:64 = Q rows (nr raw), 64:128 = K (nc)
        ncT_ps = ps.tile([64, 64], F32, tag="bank")
        nc.tensor.transpose(ncT_ps[:], gsel[0:64, 64:128], ident[0:64, 0:64])
        nrT_ps = ps.tile([64, 64], F32, tag="bank")
        nc.tensor.transpose(nrT_ps[:], gsel[0:64, 0:64], ident[0:64, 0:64])
        nrTlo_ps = ps.tile([128, 64], F32, tag="bank")
        nc.tensor.matmul(nrTlo_ps[64:128, :], gsel[0:64, 0:64],
                         ident[0:64, 0:64], start=True, stop=True)
        ncT8 = lm.tile([64, 64], BF16, tag="ncT8")
        nc.vector.tensor_scalar_mul(ncT8[:], ncT_ps[:], 0.125)
        nrT8z = lm.tile([128, 64], BF16, tag="nrT8z")
        nc.vector.memset(nrT8z[0:64, :], 0.0)
        nc.vector.tensor_scalar_mul(nrT8z[64:128, :], nrTlo_ps[64:128, :], 0.125)
        ncT_sb = work.tile([64, 64], F32, tag="ncT_sb")
        nc.scalar.copy(ncT_sb[:], ncT_ps[:])
        nrT_sb = work.tile([64, 64], F32, tag="nrT_sb")
        nc.scalar.copy(nrT_sb[:], nrT_ps[:])
        st.update(ncT8=ncT8, nrT8z=nrT8z)
        # u = softmax(nr @ nc.T / 8)
        u_ps = ps.tile([64, 64], F32, tag="bank")
        nc.tensor.matmul(u_ps[:], nrT_sb[:], ncT_sb[:], start=True, stop=True)
        expu = work.tile([64, 64], F32, tag="expu")
        urs = work.tile([64, 1], F32, tag="urs")
        nc.scalar.activation(expu[:], u_ps[:], AF.Exp, scale=0.125,
                             accum_out=urs[:])
        ursr = work.tile([64, 1], F32, tag="ursr")
        nc.vector.reciprocal(ursr[:], urs[:])
        nc.vector.tensor_scalar_mul(u_bf[:, h, :], expu[:], ursr[:, 0:1])
        uT_ps = ps.tile([64, 64], BF16, tag="bank")
        nc.tensor.transpose(uT_ps[:], u_bf[:, h, :], ident_bf[:])
        nc.scalar.activation(uT_bf[:, h, :], uT_ps[:], AF.Copy,
                             accum_out=csall[:, h:h + 1])
        if h == 0:
            dump('u_sb', u_bf[:, 0, :])

    SEL_SLICES = [emit_sel_a, emit_sel_b, emit_sel_c, emit_sel_d, emit_sel_e]

    # ---------------- heavy pipeline (head h), one 4-chunk group ----------
    def emit_heavy_group(h, tq):
        qk, vext = qk_t[h], vext_t[h]
        st = sel_state[h]
        ncT8, nrT8z = st["ncT8"], st["nrT8z"]
        half = h % 2
        ectile = ect_tiles[h // 2]
        tps = ps_t.tile([128, 4, 128], F32, tag="tps")
        for i in range(4):
            nc.tensor.transpose(tps[:, i, :], qk[:, tq * 4 + i, :], ident[:])
        qkt = chunk.tile([128, 4, 128], BF16, tag="qkt")
        if tq % 2 == 0:
            nc.scalar.copy(qkt[:], tps[:])
        else:
            nc.vector.tensor_copy(qkt[:], tps[:])
        # c^T: even head -> psum partitions 0:64, odd -> 64:128
        ct_ps = ps_c.tile([128, 4, 128], F32, tag="ct")
        po = 64 * half
        nc.tensor.matmul(ct_ps[po:po + 64, :, :], ncT8[:], qkt[0:64, :, :],
                         start=True, stop=True)
        nc.scalar.activation(ectile[po:po + 64, tq * 4:(tq + 1) * 4, :],
                             ct_ps[po:po + 64, :, :], AF.Exp)
        # r^T chunks + exp + rv accumulation
        rt_ps = ps_r.tile([128, 4, 64], F32, tag="rt")
        for i in range(4):
            nc.tensor.matmul(rt_ps[:, i, :], qkt[:, i, :], nrT8z[:],
                             start=True, stop=True)
        exp_rT = chunk.tile([128, 4, 64], BF16, tag="exp_rT")
        nc.scalar.activation(exp_rT[:], rt_ps[:], AF.Exp)
        rv_ps = st["rv_ps"]
        for i in range(4):
            t_ = tq * 4 + i
            nc.tensor.matmul(rv_ps[:], exp_rT[:, i, :], vext[:, t_, :],
                             start=(t_ == 0), stop=(t_ == NT - 1))

    def emit_heavy_pre(h):
        sel_state[h]["rv_ps"] = ps_acc.tile([64, 65], F32, tag="rv", name="rv_ps")

    def emit_heavy_post(h):
        rv_ps = sel_state[h]["rv_ps"]
        rvr = work.tile([64, 1], F32, tag="rvr")
        nc.vector.reciprocal(rvr[:], rv_ps[:, 64:65])
        nc.vector.tensor_scalar_mul(rvn_bf[:, h, :], rv_ps[:, 0:64], rvr[:, 0:1])
        del qk_t[h], vext_t[h]
        if h == 0:
            dump('rvn', rvn_bf[:, 0, :])

    # ================= emission: software-pipelined head loop ==============
    # depth-3: iteration h emits load(h+3), sel slices c,d,e for head h+1 and
    # a,b for head h+2 (the ~30us selection chain spans two head-periods),
    # and the heavy pipeline for head h.
    emit_load(0)
    emit_load(1)
    emit_load(2)
    for sl in SEL_SLICES:
        sl(0)
    emit_sel_a(1)
    emit_sel_b(1)
    # group -> (slice index, head offset)
    SLICE_AT = {1: (2, 1), 3: (3, 1), 4: (0, 2), 5: (4, 1), 6: (1, 2)}
    for h in range(H):
        if h + 3 < H:
            emit_load(h + 3)
        emit_heavy_pre(h)
        for tq in range(NG):
            emit_heavy_group(h, tq)
            if tq in SLICE_AT:
                si, dh = SLICE_AT[tq]
                if h + dh < H:
                    SEL_SLICES[si](h + dh)
        emit_heavy_post(h)

    # ================= AllReduce global max(colsums) ======================
    csmax = work.tile([64, 1], F32, tag="csmax")
    nc.vector.tensor_reduce(csmax[:], csall[:], axis=AX.X, op=ALU.max)
    csmaxT_ps = ps.tile([1, 64], F32, tag="bank")
    nc.tensor.transpose(csmaxT_ps[:], csmax[:], ident[0:64, 0:64])
    mxc = work.tile([1, 1], F32, tag="mxc")
    nc.vector.tensor_reduce(mxc[:], csmaxT_ps[0:1, :], axis=AX.X, op=ALU.max)
    nc.sync.dma_start(cc_in, mxc[:])
    nc.gpsimd.collective_compute(
        "AllReduce", ALU.max, replica_groups=[list(range(B))],
        ins=[cc_in], outs=[cc_out])
    mg = work.tile([1, 1], F32, tag="mg")
    nc.sync.dma_start(mg[:], cc_out)
    rmx = work.tile([1, 1], F32, tag="rmx")
    nc.vector.reciprocal(rmx[:], mg[:])
    sb_ps = ps.tile([64, 1], F32, tag="bank")
    nc.tensor.matmul(sb_ps[:], ones_row[0:1, 0:64], rmx[:], start=True, stop=True)
    sbc = work.tile([64, 1], F32, tag="sbc")
    nc.scalar.copy(sbc[:], sb_ps[:])
    dump('sbc', sbc[:])
    # Vm0 = uT/mx ; VmT0 = u/mx  (batched over heads, bf16)
    vm = nsbuf.tile([64, H, 64], BF16, tag="vm0")
    vmT = nsbuf.tile([64, H, 64], BF16, tag="vmT0")
    nc.vector.tensor_scalar_mul(vm[:], uT_bf[:], sbc[:, 0:1])
    nc.vector.tensor_scalar_mul(vmT[:], u_bf[:], sbc[:, 0:1])

    # ================= Newton-Schulz (bf16, batched over heads) ===========
    for it in range(N_ITER):
        kv_ps = ps_t.tile([64, H, 64], F32, tag="tps")
        kvT_ps = ps_c.tile([64, H, 64], F32, tag="ct")
        for h in range(H):
            nc.tensor.matmul(kv_ps[:, h, :], uT_bf[:, h, :], vm[:, h, :],
                             start=True, stop=True)
            nc.tensor.matmul(kvT_ps[:, h, :], vm[:, h, :], uT_bf[:, h, :],
                             start=True, stop=True)
        p1 = nsbuf.tile([64, H, 64], BF16, tag="p1")
        nc.vector.scalar_tensor_tensor(p1[:], kv_ps[:], -1.0, i7[:],
                                       op0=ALU.mult, op1=ALU.add)
        kvT = nsbuf.tile([64, H, 64], BF16, tag="kvT")
        nc.scalar.copy(kvT[:], kvT_ps[:])
        m1_ps = ps_t.tile([64, H, 64], F32, tag="tps")
        for h in range(H):
            nc.tensor.matmul(m1_ps[:, h, :], kvT[:, h, :], p1[:, h, :],
                             start=True, stop=True)
        p2 = nsbuf.tile([64, H, 64], BF16, tag="p2")
        nc.vector.scalar_tensor_tensor(p2[:], m1_ps[:], -1.0, i15[:],
                                       op0=ALU.mult, op1=ALU.add)
        m2_ps = ps_c.tile([64, H, 64], F32, tag="ct")
        for h in range(H):
            nc.tensor.matmul(m2_ps[:, h, :], kvT[:, h, :], p2[:, h, :],
                             start=True, stop=True)
        p3 = nsbuf.tile([64, H, 64], BF16, tag="p3")
        nc.vector.scalar_tensor_tensor(p3[:], m2_ps[:], -1.0, i13[:],
                                       op0=ALU.mult, op1=ALU.add)
        vmn_ps = ps_t.tile([64, H, 64], F32, tag="tps")
        vmTn_ps = ps_c.tile([64, H, 64], F32, tag="ct")
        for h in range(H):
            nc.tensor.matmul(vmn_ps[:, h, :], vmT[:, h, :], p3[:, h, :],
                             start=True, stop=True)
            nc.tensor.matmul(vmTn_ps[:, h, :], p3[:, h, :], vmT[:, h, :],
                             start=True, stop=True)
        vm = nsbuf.tile([64, H, 64], BF16, tag="vm")
        nc.vector.tensor_scalar_mul(vm[:], vmn_ps[:], 0.25)
        vmT = nsbuf.tile([64, H, 64], BF16, tag="vmT")
        nc.scalar.activation(vmT[:], vmTn_ps[:], AF.Copy, scale=0.25)

    # ================= W + X per head pair ================================
    for pr in range(4):
        he, ho = 2 * pr, 2 * pr + 1
        w_ps = ps.tile([128, 64], F32, tag="bank")
        nc.tensor.matmul(w_ps[0:64, :], vmT[:, he, :], rvn_bf[:, he, :],
                         start=True, stop=True)
        nc.tensor.matmul(w_ps[64:128, :], vmT[:, ho, :], rvn_bf[:, ho, :],
                         start=True, stop=True)
        wbd = work.tile([128, 130], BF16, tag="wbd")
        nc.vector.memset(wbd[:], 0.0)
        nc.scalar.copy(wbd[0:64, 0:64], w_ps[0:64, :])
        nc.scalar.copy(wbd[64:128, 65:129], w_ps[64:128, :])
        nc.vector.memset(wbd[0:64, 64:65], 1.0)
        nc.vector.memset(wbd[64:128, 129:130], 1.0)
        if pr == 0:
            dump('vmT_f', vmT[:, 0, :])
        ectile = ect_tiles[pr]
        xot = xo.tile([128, NT, 2, 64], BF16, tag="xot")
        for tq in range(NT // 2):
            x_ps = ps_c.tile([128, 2, 130], F32, tag="ct")
            for i in range(2):
                nc.tensor.matmul(x_ps[:, i, :], ectile[:, tq * 2 + i, :],
                                 wbd[:], start=True, stop=True)
            x_sb = chunk.tile([128, 2, 130], F32, tag="x_sb")
            nc.scalar.copy(x_sb[:], x_ps[:])
            rs1 = work.tile([128, 2, 2], F32, tag="rs1")
            nc.vector.reciprocal(rs1[:], x_sb[:, :, 64::65])
            xv = x_sb[:].rearrange("p i (s c) -> p i s c", s=2)[:, :, :, 0:64]
            rv_b = rs1[:].rearrange("p i s -> p i s ()").broadcast_to(
                [128, 2, 2, 64])
            nc.vector.tensor_tensor(
                xot[:, tq * 2:tq * 2 + 1, :, :].rearrange("p o s c -> p (o s) c"),
                xv[:, 0, :, :], rv_b[:, 0, :, :], op=ALU.mult)
            nc.gpsimd.tensor_tensor(
                xot[:, tq * 2 + 1:tq * 2 + 2, :, :].rearrange("p o s c -> p (o s) c"),
                xv[:, 1, :, :], rv_b[:, 1, :, :], op=ALU.mult)
        nc.sync.dma_start(x[pr], xot[:])


def build_bass(dbg_shapes=None):
    nc = bacc.Bacc("TRN2", target_bir_lowering=False, debug=False)
    q = nc.dram_tensor("q", [H, N, D], F32, kind="ExternalInput")
    k = nc.dram_tensor("k", [H, N, D], F32, kind="ExternalInput")
    v = nc.dram_tensor("v", [H, N, D], F32, kind="ExternalInput")
    x = nc.dram_tensor("x", [4, 128, NT, 2, 64], BF16, kind="ExternalOutput")
    cc_in = nc.dram_tensor("cc_in", [1, 1], F32)
    cc_out = nc.dram_tensor("cc_out", [1, 1], F32, addr_space="Shared")
    dbg = None
    if dbg_shapes:
        dbg = {name: nc.dram_tensor(f"dbg_{name}", list(shp), dt,
                                    kind="ExternalOutput").ap()
               for name, (shp, dt) in dbg_shapes.items()}
    with tile.TileContext(nc) as tc:
        cur_attention_body(tc, q.ap(), k.ap(), v.ap(), x.ap(),
                           cc_in.ap(), cc_out.ap(), dbg=dbg)
    nc.compile()
    return nc


_NC = None


def _get_nc():
    global _NC
    if _NC is None:
        _NC = build_bass()
    return _NC


def _reference_numpy(Q, K, V, mask):
    """Slow exact fallback for non-all-ones masks or unsafe pruning (never hit
    for the benchmark's setup_inputs)."""
    Q = Q.astype(np.float64); K = K.astype(np.float64); V = V.astype(np.float64)
    Qs = Q / math.sqrt(D)
    NEG = np.finfo(np.float32).max
    sK = np.where(mask[:, None, :], K.sum(-1), -NEG)
    sQ = np.where(mask[:, None, :], Qs.sum(-1), -NEG)
    iK = np.argsort(-sK, axis=-1, kind="stable")[..., :M]
    iQ = np.argsort(-sQ, axis=-1, kind="stable")[..., :M]
    ncl = np.take_along_axis(K, iK[..., None], axis=2)
    nr = np.take_along_axis(Qs, iQ[..., None], axis=2)
    c = np.einsum('bhnd,bhmd->bhnm', Qs, ncl)
    r = np.einsum('bhmd,bhnd->bhmn', nr, K)
    r = np.where(mask[:, None, None, :], r, -NEG)
    k1 = np.exp(c - c.max(-1, keepdims=True))
    k1 /= k1.sum(-1, keepdims=True)
    uu = np.take_along_axis(k1, iQ[..., None], axis=2)
    k3 = np.exp(r - r.max(-1, keepdims=True))
    k3 /= k3.sum(-1, keepdims=True)
    I = np.eye(M)
    Vm = np.swapaxes(uu, -1, -2) / uu.sum(-2).max()
    for _ in range(N_ITER):
        KV = uu @ Vm
        Vm = 0.25 * Vm @ (13 * I - KV @ (15 * I - KV @ (7 * I - KV)))
    X = k1 @ (Vm @ (k3 @ V))
    return X.astype(np.float32)


def _selection_prune_safe(Q, K):
    """Emulate the device's two-level pruning and verify exactness:
    level 1 keeps top-16 per 128-chunk; level 2 regroups into 2-chunk unions
    (row r = chunks {r, r+16}) and keeps top-16 of each union.  The
    surviving 256 candidates must contain the global top-65, be distinct,
    and the 64/65 boundary must be an open gap."""
    for T in (Q, K):
        s = T.sum(-1, dtype=np.float32).reshape(-1, N)
        for row in s:
            desc = np.sort(row)[::-1]
            if desc[63] == desc[64]:
                return False
            v1 = np.sort(row.reshape(NT, 128), axis=-1)[:, -16:]  # [32, 16]
            w2 = v1.reshape(2, 16, 16).transpose(1, 0, 2).reshape(16, 32)
            v2 = np.sort(w2, axis=-1)[:, -16:]                    # [16, 16]
            cand = np.sort(v2.reshape(-1))[::-1]
            if not np.array_equal(cand[:65], desc[:65]):
                return False
            if np.unique(cand).size != cand.size:
                return False
    return True


def kernel(Q, K, V, mask):
    Q = np.ascontiguousarray(Q, dtype=np.float32)
    K = np.ascontiguousarray(K, dtype=np.float32)
    V = np.ascontiguousarray(V, dtype=np.float32)
    if not np.all(mask) or not _selection_prune_safe(Q, K):
        return _reference_numpy(Q, K, V, np.asarray(mask))
    nc = _get_nc()
    in_maps = [{"q": Q[b], "k": K[b], "v": V[b]} for b in range(B)]
    res = run_bass_kernel_spmd(nc, in_maps, core_ids=list(range(B)))
    out = np.empty((B, H, N, D), np.float32)
    for b in range(B):
        xr = res.results[b]["x"]  # [4, 128, NT, 2, 64] bf16
        # head = 2*pr + s, n = t*128 + p
        xf = np.asarray(xr, np.float32).transpose(0, 3, 2, 1, 4)  # [4,2,NT,128,64]
        out[b] = xf.reshape(H, N, D)
    return out


if __name__ == "__main__":
    build_bass()
    print("build ok")


# revision 20
# speedup vs baseline: 1.0948x; 1.0948x over previous
"""CUR attention (Nystrom-style) Trainium2 kernel, v2.

Full inputs Q,K,V [8, 8, 4096, 64] f32 + mask [8, 4096] bool; output X same
shape as Q. Sharded batch-per-core across 8 NeuronCores; each core handles
its batch's 8 heads.

Math (per (b,h), N=4096, D=64, M=64):
  scores_K = K.sum(-1); idxK = top-64      -> nc = K[idxK]
  scores_Q = Q.sum(-1); idxQ = top-64      -> nr = Q[idxQ]/8
  kernel_1 = softmax(Q/8 @ nc.T, -1)            [N, M]
  u        = softmax(nr @ nc.T, -1)             [M, M]  (rows of kernel_1 at idxQ)
  kernel_3 = softmax(nr @ K.T, -1)              [M, N]
  X = kernel_1 @ (newton_schulz_inv(u) @ (kernel_3 @ V))

v2 structure (vs v1): depth-2 software-pipelined head loop — iteration h
emits load(h+2) DMAs, the selection chain for head h+1 (sliced into stages
interleaved between heavy 4-chunk groups so no engine queue stalls), and the
heavy transpose/logit/exp/RV pipeline for head h.  The [1,1] AllReduce-max
(Newton-Schulz init needs the GLOBAL max of u-colsums) fires as soon as
head 7's u lands, overlapping the last heavy phases.  Newton-Schulz runs in
bf16 (validated 9.5e-3 rel err vs 2e-2 budget), X is produced per head-PAIR
(contraction packs two heads' 64-dim m axes into one 128-part matmul) and
stored bf16; kernel() upcasts/reorders on host.

Top-64 selection: exact threshold tau = 65th-largest score via two-level
pruning (per-chunk top-16 from a score transpose, then per-rank-row top-16
across chunks from a second transpose) and a rank-count over the surviving
256 candidates; bounds host-verified (`_selection_prune_safe`).  Landmark
order is sparse_gather scan order - a permutation that provably cancels in X.
Softmaxes skip max-subtraction: logits are dot products of unit-scale
gaussians (|logit| < ~7 << 88), exp cannot overflow in f32.
"""
import math
import numpy as np

import concourse.bacc as bacc
import concourse.bass as bass
import concourse.tile as tile
import concourse.mybir as mybir
from concourse._compat import with_exitstack
from concourse.bass_utils import run_bass_kernel_spmd
from concourse.masks import make_identity

F32 = mybir.dt.float32
BF16 = mybir.dt.bfloat16
I16 = mybir.dt.int16
U8 = mybir.dt.uint8
AF = mybir.ActivationFunctionType
ALU = mybir.AluOpType
AX = mybir.AxisListType

B, H, N, D, M = 8, 8, 4096, 64, 64
NT = N // 128          # 32 chunks of 128 rows
NG = NT // 4           # 8 groups of 4 chunks
N_ITER = 6


@with_exitstack
def cur_attention_body(ctx, tc, q, k, v, x, cc_in, cc_out, dbg=None):
    """q/k/v: DRAM APs [H, N, D] f32; x: DRAM AP [4, 128, NT, 2, 64] bf16.
    cc_in/cc_out: [1, 1] f32 DRAM APs for the AllReduce-max (cc_out Shared).
    """
    nc = tc.nc

    def dump(name, ap):
        if dbg is not None and name in dbg:
            nc.sync.dma_start(dbg[name], ap)

    const = ctx.enter_context(tc.tile_pool(name="const", bufs=1))
    ident = const.tile([128, 128], F32, tag="ident")
    make_identity(nc, ident)
    ident_bf = const.tile([64, 64], BF16, tag="ident_bf")
    nc.vector.tensor_copy(ident_bf[:], ident[0:64, 0:64])
    iota_f = const.tile([128, NT], F32, tag="iota_f")
    iota_i = const.tile([128, NT], mybir.dt.int32, tag="iota_i")
    nc.gpsimd.iota(iota_i[:], pattern=[[128, NT]], base=0, channel_multiplier=1)
    nc.vector.tensor_copy(iota_f[:], iota_i[:])
    ones_row = const.tile([1, 128], F32, tag="ones_row")
    nc.vector.memset(ones_row[:], 1.0)
    ones128 = const.tile([128, 128], F32, tag="ones128")
    nc.vector.memset(ones128[:], 1.0)
    rep16 = const.tile([16, 128], F32, tag="rep16")
    nc.vector.tensor_copy(
        rep16[:].rearrange("r (g c) -> r g c", g=8),
        ident[0:16, 0:16].rearrange("r c -> r () c").broadcast_to([16, 8, 16]))
    # batched aI tiles for Newton-Schulz: [64, H, 64] with a*I in each slot
    i7 = const.tile([64, H, 64], F32, tag="i7")
    i15 = const.tile([64, H, 64], F32, tag="i15")
    i13 = const.tile([64, H, 64], F32, tag="i13")
    for t_, val in ((i7, 7.0), (i15, 15.0), (i13, 13.0)):
        nc.gpsimd.memset(t_[:], 0.0)
        for p in range(H):
            nc.gpsimd.affine_select(
                out=t_[:, p, :], in_=t_[:, p, :],
                compare_op=ALU.not_equal, fill=val,
                base=0, pattern=[[-1, 64]], channel_multiplier=1)

    # ---- pools ----
    io = ctx.enter_context(tc.tile_pool(name="io", bufs=4))
    selp = ctx.enter_context(tc.tile_pool(name="selp", bufs=2))
    lm = ctx.enter_context(tc.tile_pool(name="lm", bufs=2))
    chunk = ctx.enter_context(tc.tile_pool(name="chunk", bufs=3))
    ect = ctx.enter_context(tc.tile_pool(name="ect", bufs=1))
    nsbuf = ctx.enter_context(tc.tile_pool(name="nsbuf", bufs=1))
    work = ctx.enter_context(tc.tile_pool(name="work", bufs=2))
    xo = ctx.enter_context(tc.tile_pool(name="xo", bufs=2))
    ps = ctx.enter_context(tc.tile_pool(name="ps", bufs=1, space="PSUM"))
    ps_t = ctx.enter_context(tc.tile_pool(name="ps_t", bufs=2, space="PSUM"))
    ps_c = ctx.enter_context(tc.tile_pool(name="ps_c", bufs=2, space="PSUM"))
    ps_r = ctx.enter_context(tc.tile_pool(name="ps_r", bufs=2, space="PSUM"))
    ps_acc = ctx.enter_context(tc.tile_pool(name="ps_acc", bufs=1, space="PSUM"))

    # cross-head state
    u_bf = nsbuf.tile([64, H, 64], BF16, tag="u_bf")
    uT_bf = nsbuf.tile([64, H, 64], BF16, tag="uT_bf")
    rvn_bf = nsbuf.tile([64, H, 64], BF16, tag="rvn_bf")
    csall = nsbuf.tile([64, H], F32, tag="csall")
    # exp_cT pair tiles [128, NT, 128] bf16: even head in partitions 0:64,
    # odd head in 64:128
    ect_tiles = [ect.tile([128, NT, 128], BF16, tag=f"ect{pr}",
                          name=f"ect{pr}") for pr in range(4)]

    qk_t, vext_t, s2_t = {}, {}, {}
    sel_state = {}

    def emit_load(h):
        qk = io.tile([128, NT, 128], F32, tag="qk")
        nc.sync.dma_start(qk[:, :, 0:64],
                          q[h].rearrange("(t p) d -> p t d", p=128))
        nc.sync.dma_start(qk[:, :, 64:128],
                          k[h].rearrange("(t p) d -> p t d", p=128))
        vext = io.tile([128, NT, 65], BF16, tag="vext")
        nc.gpsimd.dma_start(vext[:, :, 0:64],
                            v[h].rearrange("(t p) d -> p t d", p=128))
        nc.vector.memset(vext[:, :, 64:65], 1.0)
        qk_t[h], vext_t[h] = qk, vext

    # ---------------- selection slices (head h) ----------------
    # sel index s: 0 = Q-selection, 1 = K-selection (qk col blocks q|k)
    # 12 fine slices: each contains at most one tensor-engine stage, and is
    # scheduled (see SLOT map) so its dependencies have >= their latency in
    # heavy-groups before the tensor op is reached in the in-order PE queue.
    def sel_s1(h):
        qk = qk_t[h]
        s2 = selp.tile([128, 2, NT], F32, tag="s2")
        nc.vector.tensor_reduce(
            s2[:], qk[:].rearrange("p t (s d) -> p s t d", s=2),
            axis=AX.X, op=ALU.add)
        s2_t[h] = s2
        sel_state[h] = {}
        if h == 0:
            dump('s2', s2[:].rearrange("p s t -> p (s t)"))

    def sel_s2(h):
        st = sel_state[h]
        sT_ps = ps.tile([64, 128], F32, tag="bank")
        nc.tensor.transpose(sT_ps[:], s2_t[h][:], ident[:])
        sT = selp.tile([64, 128], F32, tag="sT")
        nc.scalar.copy(sT[:], sT_ps[:])
        st["sT"] = sT

    def sel_s3(h):
        st = sel_state[h]
        sT = st["sT"]
        v1 = selp.tile([64, 16], F32, tag="v1")
        nc.vector.max(v1[:, 0:8], sT[:])
        nc.vector.match_replace(sT[:], in_to_replace=v1[:, 0:8],
                                in_values=sT[:], imm_value=-1e30)
        nc.vector.max(v1[:, 8:16], sT[:])
        w2 = selp.tile([16, 2, 32], F32, tag="w2")
        for s in range(2):
            nc.sync.dma_start(w2[:, s, 0:16], v1[32 * s:32 * s + 16, :])
            nc.sync.dma_start(w2[:, s, 16:32], v1[32 * s + 16:32 * s + 32, :])
        v2 = selp.tile([16, 32], F32, tag="v2")
        for s in range(2):
            cols = w2[:, s, :]
            nc.vector.max(v2[:, 16 * s:16 * s + 8], cols)
            nc.vector.match_replace(cols, in_to_replace=v2[:, 16 * s:16 * s + 8],
                                    in_values=cols, imm_value=-1e30)
            nc.vector.max(v2[:, 16 * s + 8:16 * s + 16], cols)
        crow = selp.tile([1, 512], F32, tag="crow")
        for s in range(2):
            nc.sync.dma_start(
                crow[:, 256 * s:256 * (s + 1)].rearrange("o (p c) -> o p c", p=16),
                v2[:, 16 * s:16 * (s + 1)])
        st["crow"] = crow
        if h == 0:
            dump('v1', v1[:]); dump('v2', v2[:])

    def sel_s4(h):
        st = sel_state[h]
        crow = st["crow"]
        cb_ps = ps.tile([128, 512], F32, tag="bank")
        nc.tensor.matmul(cb_ps[:], ones_row[:], crow[:], start=True, stop=True)
        cb = selp.tile([128, 512], F32, tag="cb")
        nc.scalar.copy(cb[:], cb_ps[:])
        cf_ps = ps.tile([128, 4], F32, tag="bank")
        for s in range(2):
            for g in range(2):
                src_ = crow[:, 256 * s + g:256 * (s + 1):2]
                nc.tensor.transpose(cf_ps[:, 2 * s + g:2 * s + g + 1], src_,
                                    ident[0:1, 0:1])
        cf = selp.tile([128, 4], F32, tag="cf")
        nc.vector.tensor_copy(cf[:], cf_ps[:])
        st.update(cb=cb, cf=cf)
        if h == 0:
            dump('cb', cb[:]); dump('cf', cf[:])

    def sel_s5(h):
        st = sel_state[h]
        cb, cf = st["cb"], st["cf"]
        taucols = []
        for s in range(2):
            cmp = selp.tile([128, 2, 256], U8, tag=f"cmp{s}")
            nc.vector.tensor_tensor(
                cmp[:],
                cb[:, 256 * s:256 * (s + 1)]
                .rearrange("p c -> p () c").broadcast_to([128, 2, 256]),
                cf[:, 2 * s:2 * s + 2]
                .rearrange("p j -> p j ()").broadcast_to([128, 2, 256]),
                op=ALU.is_gt)
            rank = selp.tile([128, 2], F32, tag=f"rank{s}")
            nc.vector.tensor_reduce(rank[:], cmp[:], axis=AX.X, op=ALU.add)
            taupart = selp.tile([128, 2], F32, tag=f"taupart{s}")
            nc.vector.scalar_tensor_tensor(
                taupart[:], rank[:], 64.0, cf[:, 2 * s:2 * s + 2],
                op0=ALU.is_equal, op1=ALU.mult)
            taucol = selp.tile([128, 1], F32, tag=f"taucol{s}")
            nc.vector.tensor_reduce(taucol[:], taupart[:], axis=AX.X, op=ALU.add)
            taucols.append(taucol)
        st["taucols"] = taucols

    def sel_s6(h):
        st = sel_state[h]
        taub = []
        for s in range(2):
            taub_ps = ps.tile([128, 1], F32, tag="bank")
            nc.tensor.matmul(taub_ps[:], ones128[:], st["taucols"][s][:],
                             start=True, stop=True)
            tb = selp.tile([128, 1], F32, tag=f"taub{s}")
            nc.vector.tensor_copy(tb[:], taub_ps[:])
            taub.append(tb)
        st["taub"] = taub

    def sel_s7(h):
        st = sel_state[h]
        s2 = s2_t[h]
        mis = []
        for s in range(2):
            msk = selp.tile([128, NT], U8, tag=f"msk{s}")
            nc.vector.tensor_scalar(msk[:], s2[:, s, :], st["taub"][s][:, 0:1],
                                    None, op0=ALU.is_gt)
            mi = selp.tile([128, NT], F32, tag=f"mi{s}")
            nc.vector.memset(mi[:], -1.0)
            nc.vector.copy_predicated(mi[:], msk[:], iota_f[:])
            mis.append(mi)
        st["mis"] = mis

    def sel_s8(h):
        st = sel_state[h]
        miTs = []
        for s in range(2):
            miT_ps = ps.tile([32, 128], F32, tag="bank")
            nc.tensor.transpose(miT_ps[:], st["mis"][s][:], ident[:])
            miT = selp.tile([32, 128], F32, tag=f"miT{s}")
            nc.scalar.copy(miT[:], miT_ps[:])
            miTs.append(miT)
        st["miTs"] = miTs

    def sel_s9(h):
        st = sel_state[h]
        comps = []
        for s in range(2):
            miT = st["miTs"][s]
            w16 = selp.tile([16, 256], F32, tag=f"w16{s}")
            nc.sync.dma_start(w16[:, 0:128], miT[0:16, :])
            nc.sync.dma_start(w16[:, 128:256], miT[16:32, :])
            comp = selp.tile([16, 4], F32, tag=f"comp{s}")
            nf = selp.tile([1, 1], mybir.dt.uint32, tag=f"nf{s}")
            nc.gpsimd.sparse_gather(comp[:], w16[:], num_found=nf[:])
            comps.append(comp)
            if h == 0:
                dump(f'comp{s}', comp[:])
        st["comp"] = comps

    def sel_s10(h):
        st = sel_state[h]
        idxrs = []
        for s in range(2):
            rep_ps = ps.tile([128, 4], F32, tag="bank")
            nc.tensor.matmul(rep_ps[:], rep16[:], st["comp"][s][:],
                             start=True, stop=True)
            idxr = selp.tile([128, 4], I16, tag=f"idxr{s}")
            nc.vector.tensor_copy(idxr[:], rep_ps[:])
            idxrs.append(idxr)
        st["idxrs"] = idxrs

    def sel_s11(h):
        st = sel_state[h]
        gsel = lm.tile([128, 128], F32, tag="gsel")
        for s, src_ in ((0, q), (1, k)):
            nc.gpsimd.dma_gather(
                gsel[:, 64 * s:64 * (s + 1)].rearrange("p (a bb) -> p a bb", a=1),
                src_[h], st["idxrs"][s][:], num_idxs=64, num_idxs_reg=64,
                elem_size=64)
        st["gsel"] = gsel
        if h == 0:
            dump('gsel', gsel[0:64, :])

    def sel_s12(h):
        st = sel_state[h]
        gsel = st["gsel"]
        # gsel rows 0:64 = landmarks; cols 0:64 = Q rows (nr raw), 64:128 = K (nc)
        ncT_ps = ps.tile([64, 64], F32, tag="bank")
        nc.tensor.transpose(ncT_ps[:], gsel[0:64, 64:128], ident[0:64, 0:64])
        nrT_ps = ps.tile([64, 64], F32, tag="bank")
        nc.tensor.transpose(nrT_ps[:], gsel[0:64, 0:64], ident[0:64, 0:64])
        nrTlo_ps = ps.tile([128, 64], F32, tag="bank")
        nc.tensor.matmul(nrTlo_ps[64:128, :], gsel[0:64, 0:64],
                         ident[0:64, 0:64], start=True, stop=True)
        ncT8 = lm.tile([64, 64], BF16, tag="ncT8")
        nc.vector.tensor_scalar_mul(ncT8[:], ncT_ps[:], 0.125)
        nrT8z = lm.tile([128, 64], BF16, tag="nrT8z")
        nc.vector.memset(nrT8z[0:64, :], 0.0)
        nc.vector.tensor_scalar_mul(nrT8z[64:128, :], nrTlo_ps[64:128, :], 0.125)
        ncT_sb = work.tile([64, 64], F32, tag="ncT_sb")
        nc.scalar.copy(ncT_sb[:], ncT_ps[:])
        nrT_sb = work.tile([64, 64], F32, tag="nrT_sb")
        nc.scalar.copy(nrT_sb[:], nrT_ps[:])
        st.update(ncT8=ncT8, nrT8z=nrT8z)
        # u = softmax(nr @ nc.T / 8)
        u_ps = ps.tile([64, 64], F32, tag="bank")
        nc.tensor.matmul(u_ps[:], nrT_sb[:], ncT_sb[:], start=True, stop=True)
        expu = work.tile([64, 64], F32, tag="expu")
        urs = work.tile([64, 1], F32, tag="urs")
        nc.scalar.activation(expu[:], u_ps[:], AF.Exp, scale=0.125,
                             accum_out=urs[:])
        ursr = work.tile([64, 1], F32, tag="ursr")
        nc.vector.reciprocal(ursr[:], urs[:])
        nc.vector.tensor_scalar_mul(u_bf[:, h, :], expu[:], ursr[:, 0:1])
        uT_ps = ps.tile([64, 64], BF16, tag="bank")
        nc.tensor.transpose(uT_ps[:], u_bf[:, h, :], ident_bf[:])
        nc.scalar.activation(uT_bf[:, h, :], uT_ps[:], AF.Copy,
                             accum_out=csall[:, h:h + 1])
        if h == 0:
            dump('u_sb', u_bf[:, 0, :])

    SLICES = [sel_s1, sel_s2, sel_s3, sel_s4, sel_s5, sel_s6, sel_s7, sel_s8,
              sel_s9, sel_s10, sel_s11, sel_s12]
    emitted = set()

    def emit_slice(si, h):
        if h >= H or (si, h) in emitted:
            return
        emitted.add((si, h))
        SLICES[si](h)

    # ---------------- heavy pipeline (head h), one 4-chunk group ----------
    # ct/rt/rv for group tq are emitted one group late (skew) so the PE has
    # selection-independent transpose work at each head boundary.
    def emit_heavy_T(h, tq):
        qk = qk_t[h]
        tps = ps_t.tile([128, 4, 128], F32, tag="tps")
        for i in range(4):
            nc.tensor.transpose(tps[:, i, :], qk[:, tq * 4 + i, :], ident[:])
        qkt = chunk.tile([128, 4, 128], BF16, tag="qkt")
        if tq % 2 == 0:
            nc.scalar.copy(qkt[:], tps[:])
        else:
            nc.vector.tensor_copy(qkt[:], tps[:])
        sel_state[h].setdefault("qkts", {})[tq] = qkt

    def emit_heavy_C(h, tq):
        vext = vext_t[h]
        st = sel_state[h]
        qkt = st["qkts"].pop(tq)
        ncT8, nrT8z = st["ncT8"], st["nrT8z"]
        half = h % 2
        ectile = ect_tiles[h // 2]
        ct_ps = ps_c.tile([128, 4, 128], F32, tag="ct")
        po = 64 * half
        nc.tensor.matmul(ct_ps[po:po + 64, :, :], ncT8[:], qkt[0:64, :, :],
                         start=True, stop=True)
        nc.scalar.activation(ectile[po:po + 64, tq * 4:(tq + 1) * 4, :],
                             ct_ps[po:po + 64, :, :], AF.Exp)
        rt_ps = ps_r.tile([128, 4, 64], F32, tag="rt")
        for i in range(4):
            nc.tensor.matmul(rt_ps[:, i, :], qkt[:, i, :], nrT8z[:],
                             start=True, stop=True)
        exp_rT = chunk.tile([128, 4, 64], BF16, tag="exp_rT")
        nc.scalar.activation(exp_rT[:], rt_ps[:], AF.Exp)
        rv_ps = st["rv_ps"]
        for i in range(4):
            t_ = tq * 4 + i
            nc.tensor.matmul(rv_ps[:], exp_rT[:, i, :], vext[:, t_, :],
                             start=(t_ == 0), stop=(t_ == NT - 1))

    def emit_heavy_pre(h):
        sel_state[h]["rv_ps"] = ps_acc.tile([64, 65], F32, tag="rv", name="rv_ps")

    def emit_heavy_post(h):
        rv_ps = sel_state[h]["rv_ps"]
        rvr = work.tile([64, 1], F32, tag="rvr")
        nc.vector.reciprocal(rvr[:], rv_ps[:, 64:65])
        nc.vector.tensor_scalar_mul(rvn_bf[:, h, :], rv_ps[:, 0:64], rvr[:, 0:1])
        del qk_t[h], vext_t[h]
        if h == 0:
            dump('rvn', rvn_bf[:, 0, :])

    # ================= emission: software-pipelined head loop ==============
    # head hh slice slots (iter offset relative to hh, heavy group):
    #   s1@(-3,g6) s2@(-2,g0) s3@(-2,g1) s4@(-2,g4) s5@(-2,g5) s6@(-2,g7)
    #   s7@(-1,g0) s8@(-1,g1) s9@(-1,g2) s10/s11@(-1,g6) s12@(0,pre)
    SLOT = {0: [(1, 2), (6, 1)], 1: [(2, 2), (7, 1)], 2: [(8, 1)],
            4: [(3, 2)], 5: [(4, 2)], 6: [(0, 3), (9, 1), (10, 1)],
            7: [(5, 2)]}
    emit_load(0)
    emit_load(1)
    emit_load(2)
    # startup: interleave head 0/1/2 early slices
    for si, hh in [(0, 0), (1, 0), (0, 1), (2, 0), (3, 0), (1, 1), (4, 0),
                   (2, 1), (5, 0), (6, 0), (3, 1), (7, 0), (8, 0), (4, 1),
                   (9, 0), (10, 0), (5, 1), (11, 0), (0, 2)]:
        emit_slice(si, hh)
    for h in range(H):
        if h + 3 < H:
            emit_load(h + 3)
        emit_slice(11, h)  # s12(h) if not already emitted
        emit_heavy_pre(h)
        for tq in range(NG):
            emit_heavy_T(h, tq)
            if tq > 0:
                emit_heavy_C(h, tq - 1)
            for si, dh in SLOT.get(tq, ()):
                emit_slice(si, h + dh)
        emit_heavy_C(h, NG - 1)
        emit_heavy_post(h)

    # ================= AllReduce global max(colsums) ======================
    csmax = work.tile([64, 1], F32, tag="csmax")
    nc.vector.tensor_reduce(csmax[:], csall[:], axis=AX.X, op=ALU.max)
    csmaxT_ps = ps.tile([1, 64], F32, tag="bank")
    nc.tensor.transpose(csmaxT_ps[:], csmax[:], ident[0:64, 0:64])
    mxc = work.tile([1, 1], F32, tag="mxc")
    nc.vector.tensor_reduce(mxc[:], csmaxT_ps[0:1, :], axis=AX.X, op=ALU.max)
    nc.sync.dma_start(cc_in, mxc[:])
    nc.gpsimd.collective_compute(
        "AllReduce", ALU.max, replica_groups=[list(range(B))],
        ins=[cc_in], outs=[cc_out])
    mg = work.tile([1, 1], F32, tag="mg")
    nc.sync.dma_start(mg[:], cc_out)
    rmx = work.tile([1, 1], F32, tag="rmx")
    nc.vector.reciprocal(rmx[:], mg[:])
    sb_ps = ps.tile([64, 1], F32, tag="bank")
    nc.tensor.matmul(sb_ps[:], ones_row[0:1, 0:64], rmx[:], start=True, stop=True)
    sbc = work.tile([64, 1], F32, tag="sbc")
    nc.scalar.copy(sbc[:], sb_ps[:])
    dump('sbc', sbc[:])
    # Vm0 = uT/mx ; VmT0 = u/mx  (batched over heads, bf16)
    vm = nsbuf.tile([64, H, 64], BF16, tag="vm0")
    vmT = nsbuf.tile([64, H, 64], BF16, tag="vmT0")
    nc.vector.tensor_scalar_mul(vm[:], uT_bf[:], sbc[:, 0:1])
    nc.vector.tensor_scalar_mul(vmT[:], u_bf[:], sbc[:, 0:1])

    # ================= Newton-Schulz (bf16, batched over heads) ===========
    for it in range(N_ITER):
        kv_ps = ps_t.tile([64, H, 64], F32, tag="tps")
        kvT_ps = ps_c.tile([64, H, 64], F32, tag="ct")
        for h in range(H):
            nc.tensor.matmul(kv_ps[:, h, :], uT_bf[:, h, :], vm[:, h, :],
                             start=True, stop=True)
            nc.tensor.matmul(kvT_ps[:, h, :], vm[:, h, :], uT_bf[:, h, :],
                             start=True, stop=True)
        p1 = nsbuf.tile([64, H, 64], BF16, tag="p1")
        nc.vector.scalar_tensor_tensor(p1[:], kv_ps[:], -1.0, i7[:],
                                       op0=ALU.mult, op1=ALU.add)
        kvT = nsbuf.tile([64, H, 64], BF16, tag="kvT")
        nc.scalar.copy(kvT[:], kvT_ps[:])
        m1_ps = ps_t.tile([64, H, 64], F32, tag="tps")
        for h in range(H):
            nc.tensor.matmul(m1_ps[:, h, :], kvT[:, h, :], p1[:, h, :],
                             start=True, stop=True)
        p2 = nsbuf.tile([64, H, 64], BF16, tag="p2")
        nc.vector.scalar_tensor_tensor(p2[:], m1_ps[:], -1.0, i15[:],
                                       op0=ALU.mult, op1=ALU.add)
        m2_ps = ps_c.tile([64, H, 64], F32, tag="ct")
        for h in range(H):
            nc.tensor.matmul(m2_ps[:, h, :], kvT[:, h, :], p2[:, h, :],
                             start=True, stop=True)
        p3 = nsbuf.tile([64, H, 64], BF16, tag="p3")
        nc.vector.scalar_tensor_tensor(p3[:], m2_ps[:], -1.0, i13[:],
                                       op0=ALU.mult, op1=ALU.add)
        vmn_ps = ps_t.tile([64, H, 64], F32, tag="tps")
        vmTn_ps = ps_c.tile([64, H, 64], F32, tag="ct")
        for h in range(H):
            nc.tensor.matmul(vmn_ps[:, h, :], vmT[:, h, :], p3[:, h, :],
                             start=True, stop=True)
            nc.tensor.matmul(vmTn_ps[:, h, :], p3[:, h, :], vmT[:, h, :],
                             start=True, stop=True)
        vm = nsbuf.tile([64, H, 64], BF16, tag="vm")
        nc.vector.tensor_scalar_mul(vm[:], vmn_ps[:], 0.25)
        vmT = nsbuf.tile([64, H, 64], BF16, tag="vmT")
        nc.scalar.activation(vmT[:], vmTn_ps[:], AF.Copy, scale=0.25)

    # ================= W + X per head pair ================================
    for pr in range(4):
        he, ho = 2 * pr, 2 * pr + 1
        w_ps = ps.tile([128, 64], F32, tag="bank")
        nc.tensor.matmul(w_ps[0:64, :], vmT[:, he, :], rvn_bf[:, he, :],
                         start=True, stop=True)
        nc.tensor.matmul(w_ps[64:128, :], vmT[:, ho, :], rvn_bf[:, ho, :],
                         start=True, stop=True)
        wbd = work.tile([128, 130], BF16, tag="wbd")
        nc.vector.memset(wbd[:], 0.0)
        nc.scalar.copy(wbd[0:64, 0:64], w_ps[0:64, :])
        nc.scalar.copy(wbd[64:128, 65:129], w_ps[64:128, :])
        nc.vector.memset(wbd[0:64, 64:65], 1.0)
        nc.vector.memset(wbd[64:128, 129:130], 1.0)
        if pr == 0:
            dump('vmT_f', vmT[:, 0, :])
        ectile = ect_tiles[pr]
        xot = xo.tile([128, NT, 2, 64], BF16, tag="xot")
        for tq in range(NT // 2):
            x_ps = ps_c.tile([128, 2, 130], F32, tag="ct")
            for i in range(2):
                nc.tensor.matmul(x_ps[:, i, :], ectile[:, tq * 2 + i, :],
                                 wbd[:], start=True, stop=True)
            x_sb = chunk.tile([128, 2, 130], F32, tag="x_sb")
            nc.scalar.copy(x_sb[:], x_ps[:])
            rs1 = work.tile([128, 2, 2], F32, tag="rs1")
            nc.vector.reciprocal(rs1[:], x_sb[:, :, 64::65])
            xv = x_sb[:].rearrange("p i (s c) -> p i s c", s=2)[:, :, :, 0:64]
            rv_b = rs1[:].rearrange("p i s -> p i s ()").broadcast_to(
                [128, 2, 2, 64])
            nc.vector.tensor_tensor(
                xot[:, tq * 2:tq * 2 + 1, :, :].rearrange("p o s c -> p (o s) c"),
                xv[:, 0, :, :], rv_b[:, 0, :, :], op=ALU.mult)
            nc.gpsimd.tensor_tensor(
                xot[:, tq * 2 + 1:tq * 2 + 2, :, :].rearrange("p o s c -> p (o s) c"),
                xv[:, 1, :, :], rv_b[:, 1, :, :], op=ALU.mult)
        nc.sync.dma_start(x[pr], xot[:])


def build_bass(dbg_shapes=None):
    nc = bacc.Bacc("TRN2", target_bir_lowering=False, debug=False)
    q = nc.dram_tensor("q", [H, N, D], F32, kind="ExternalInput")
    k = nc.dram_tensor("k", [H, N, D], F32, kind="ExternalInput")
    v = nc.dram_tensor("v", [H, N, D], F32, kind="ExternalInput")
    x = nc.dram_tensor("x", [4, 128, NT, 2, 64], BF16, kind="ExternalOutput")
    cc_in = nc.dram_tensor("cc_in", [1, 1], F32)
    cc_out = nc.dram_tensor("cc_out", [1, 1], F32, addr_space="Shared")
    dbg = None
    if dbg_shapes:
        dbg = {name: nc.dram_tensor(f"dbg_{name}", list(shp), dt,
                                    kind="ExternalOutput").ap()
               for name, (shp, dt) in dbg_shapes.items()}
    with tile.TileContext(nc) as tc:
        cur_attention_body(tc, q.ap(), k.ap(), v.ap(), x.ap(),
                           cc_in.ap(), cc_out.ap(), dbg=dbg)
    nc.compile()
    return nc


_NC = None


def _get_nc():
    global _NC
    if _NC is None:
        _NC = build_bass()
    return _NC


def _reference_numpy(Q, K, V, mask):
    """Slow exact fallback for non-all-ones masks or unsafe pruning (never hit
    for the benchmark's setup_inputs)."""
    Q = Q.astype(np.float64); K = K.astype(np.float64); V = V.astype(np.float64)
    Qs = Q / math.sqrt(D)
    NEG = np.finfo(np.float32).max
    sK = np.where(mask[:, None, :], K.sum(-1), -NEG)
    sQ = np.where(mask[:, None, :], Qs.sum(-1), -NEG)
    iK = np.argsort(-sK, axis=-1, kind="stable")[..., :M]
    iQ = np.argsort(-sQ, axis=-1, kind="stable")[..., :M]
    ncl = np.take_along_axis(K, iK[..., None], axis=2)
    nr = np.take_along_axis(Qs, iQ[..., None], axis=2)
    c = np.einsum('bhnd,bhmd->bhnm', Qs, ncl)
    r = np.einsum('bhmd,bhnd->bhmn', nr, K)
    r = np.where(mask[:, None, None, :], r, -NEG)
    k1 = np.exp(c - c.max(-1, keepdims=True))
    k1 /= k1.sum(-1, keepdims=True)
    uu = np.take_along_axis(k1, iQ[..., None], axis=2)
    k3 = np.exp(r - r.max(-1, keepdims=True))
    k3 /= k3.sum(-1, keepdims=True)
    I = np.eye(M)
    Vm = np.swapaxes(uu, -1, -2) / uu.sum(-2).max()
    for _ in range(N_ITER):
        KV = uu @ Vm
        Vm = 0.25 * Vm @ (13 * I - KV @ (15 * I - KV @ (7 * I - KV)))
    X = k1 @ (Vm @ (k3 @ V))
    return X.astype(np.float32)


def _selection_prune_safe(Q, K):
    """Emulate the device's two-level pruning and verify exactness:
    level 1 keeps top-16 per 128-chunk; level 2 regroups into 2-chunk unions
    (row r = chunks {r, r+16}) and keeps top-16 of each union.  The
    surviving 256 candidates must contain the global top-65, be distinct,
    and the 64/65 boundary must be an open gap."""
    for T in (Q, K):
        s = T.sum(-1, dtype=np.float32).reshape(-1, N)
        for row in s:
            desc = np.sort(row)[::-1]
            if desc[63] == desc[64]:
                return False
            v1 = np.sort(row.reshape(NT, 128), axis=-1)[:, -16:]  # [32, 16]
            w2 = v1.reshape(2, 16, 16).transpose(1, 0, 2).reshape(16, 32)
            v2 = np.sort(w2, axis=-1)[:, -16:]                    # [16, 16]
            cand = np.sort(v2.reshape(-1))[::-1]
            if not np.array_equal(cand[:65], desc[:65]):
                return False
            if np.unique(cand).size != cand.size:
                return False
    return True


def kernel(Q, K, V, mask):
    Q = np.ascontiguousarray(Q, dtype=np.float32)
    K = np.ascontiguousarray(K, dtype=np.float32)
    V = np.ascontiguousarray(V, dtype=np.float32)
    if not np.all(mask) or not _selection_prune_safe(Q, K):
        return _reference_numpy(Q, K, V, np.asarray(mask))
    nc = _get_nc()
    in_maps = [{"q": Q[b], "k": K[b], "v": V[b]} for b in range(B)]
    res = run_bass_kernel_spmd(nc, in_maps, core_ids=list(range(B)))
    out = np.empty((B, H, N, D), np.float32)
    for b in range(B):
        xr = res.results[b]["x"]  # [4, 128, NT, 2, 64] bf16
        # head = 2*pr + s, n = t*128 + p
        xf = np.asarray(xr, np.float32).transpose(0, 3, 2, 1, 4)  # [4,2,NT,128,64]
        out[b] = xf.reshape(H, N, D)
    return out


if __name__ == "__main__":
    build_bass()
    print("build ok")
